# revision 6
# baseline (speedup 1.0000x reference)
import numpy as np
import ml_dtypes

import concourse.bacc as bacc
import concourse.mybir as mybir
import concourse.tile as tile
from concourse.bass_utils import run_bass_kernel_spmd

bf16 = mybir.dt.bfloat16
f32 = mybir.dt.float32
i16 = mybir.dt.int16
AF = mybir.ActivationFunctionType
ALU = mybir.AluOpType

N = 10000
E = 320000
D = 128
HID = 128
OUT = 128
H = 16
EF = 4
NG = 20
RMAX = 10.0
KV = 2 * D + EF + EF * NG
EPS = 1e-5
NCORES = 8
NLOC = N // NCORES          # 1250 nodes per core
NGRP = 10                   # groups of 128 local nodes (1250 -> 10 groups)
HSTRIPES = (N + 127) // 128  # 79

_cache = {}


def _bf(x):
    return np.asarray(x, np.float32).astype(ml_dtypes.bfloat16)


def _wrap_idx(a):
    # dma_gather idx layout: idxs[i%16, i//16], replicated over 8 Q7 cores
    a = np.asarray(a, np.int16)
    assert len(a) % 16 == 0
    w = a.reshape(-1, 16).T.copy()
    return np.tile(w, (8, 1))


def _center(W, b):
    # LN mean-subtraction is linear: fold into weights (exact)
    Wc = W - W.mean(axis=1, keepdims=True)
    bc = b - b.mean()
    return Wc, bc


def _build_program(T_total, grp_start, grp_end):
    nc = bacc.Bacc("TRN2", target_bir_lowering=False, debug=False,
                   num_devices=NCORES)
    E_pad = 128 * T_total
    T_tiles = T_total // 4

    P = {}
    def inp(name, shape, dt):
        P[name] = nc.declare_dram_parameter(name, list(shape), dt, isOutput=False)
        return P[name]

    hsb_d = inp("hsb", [128, HSTRIPES, 128], bf16)
    hlocT_d = inp("hlocT", [128, NGRP * 128], bf16)
    idd_d = inp("idd", [128, E_pad // 16], i16)
    ids_d = inp("ids", [128, E_pad // 16], i16)
    idq_d = inp("idq", [128, E_pad // 16], i16)
    dfT_d = inp("dfT", [84, E_pad], bf16)
    relN_d = inp("relN", [128, E_pad // 128, 4], f32)
    S_d = inp("S", [128, T_total, 128], bf16)
    W1a_d = inp("W1a", [84, 256], bf16)
    W1b_d = inp("W1b", [128, 256], bf16)
    W1c_d = inp("W1c", [128, 256], bf16)
    W2k_d = inp("W2k", [128, 128], bf16)
    W2v_d = inp("W2v", [128, 16], bf16)
    W1q_d = inp("W1q", [128, 128], bf16)
    W2q_d = inp("W2q", [128, 128], bf16)
    bias_d = inp("bias", [128, 8], f32)   # cols: b1k b1v b1q b2k (then pad)
    b2vbc_d = inp("b2vbc", [128, 64], f32)
    b2qbc_d = inp("b2qbc", [128, 128], f32)
    ones_d = inp("ones", [128, 1], bf16)
    blk_d = inp("blk", [128, 16], bf16)
    out_d = nc.declare_dram_parameter("o", [128, NGRP, 4], f32, isOutput=True)

    with tile.TileContext(nc) as tc:
        with (
            tc.tile_pool(name="tabs", bufs=1) as tabs,
            tc.tile_pool(name="wts", bufs=1) as wts,
            tc.tile_pool(name="io", bufs=3) as io,
            tc.tile_pool(name="work", bufs=2) as work,
            tc.tile_pool(name="psY", bufs=2, space="PSUM") as psY,
            tc.tile_pool(name="ps1", bufs=1, space="PSUM") as ps1,
        ):
            # ---- resident tables / weights ----
            hsb = tabs.tile([128, HSTRIPES, 128], bf16, tag="hsb")
            hlocT = tabs.tile([128, NGRP * 128], bf16, tag="hlocT")
            idd = tabs.tile([128, E_pad // 16], i16, tag="idd")
            ids = tabs.tile([128, E_pad // 16], i16, tag="ids")
            idq = tabs.tile([128, E_pad // 16], i16, tag="idq")
            qsb = tabs.tile([128, NGRP, 128], bf16, tag="qsb")
            osb = tabs.tile([128, NGRP, 4], f32, tag="osb")
            nc.sync.dma_start(hsb[:], hsb_d[:])
            nc.sync.dma_start(hlocT[:], hlocT_d[:])
            nc.sync.dma_start(idd[:], idd_d[:])
            nc.sync.dma_start(ids[:], ids_d[:])
            nc.sync.dma_start(idq[:], idq_d[:])
            W1a = wts.tile([84, 256], bf16, tag="w1a")
            W1b = wts.tile([128, 256], bf16, tag="w1b")
            W1c = wts.tile([128, 256], bf16, tag="w1c")
            W2k = wts.tile([128, 128], bf16, tag="w2k")
            W2v = wts.tile([128, 16], bf16, tag="w2v")
            W1q = wts.tile([128, 128], bf16, tag="w1q")
            W2q = wts.tile([128, 128], bf16, tag="w2q")
            bias = wts.tile([128, 8], f32, tag="bias")
            b2vbc = wts.tile([128, 64], f32, tag="b2vbc")
            b2qbc = wts.tile([128, 128], f32, tag="b2qbc")
            ones = wts.tile([128, 1], bf16, tag="ones")
            blk = wts.tile([128, 16], bf16, tag="blk")
            for t, d in [(W1a, W1a_d), (W1b, W1b_d), (W1c, W1c_d), (W2k, W2k_d),
                         (W2v, W2v_d), (W1q, W1q_d), (W2q, W2q_d), (bias, bias_d),
                         (b2vbc, b2vbc_d), (b2qbc, b2qbc_d), (ones, ones_d),
                         (blk, blk_d)]:
                nc.sync.dma_start(t[:], d[:])
            b1k, b1v, b1q, b2k = (bias[:, j:j + 1] for j in range(4))

            # ---- stage A: q-MLP over local nodes (layout T in, N out) ----
            yq = psY.tile([128, 512], f32, tag="yk")
            ysq_q = work.tile([128, NGRP * 128], bf16, tag="ysq_q")
            zq = work.tile([128, NGRP * 128], bf16, tag="zq")
            for s in range(3):
                w = min(512, NGRP * 128 - 512 * s)
                nc.tensor.matmul(yq[:, :w], W1q[:], hlocT[:, 512 * s:512 * s + w],
                                 start=True, stop=True)
                nc.scalar.activation(ysq_q[:, 512 * s:512 * s + w], yq[:, :w],
                                     AF.Square, bias=b1q)
                nc.vector.tensor_scalar(zq[:, 512 * s:512 * s + w], yq[:, :w],
                                        b1q, 0.0, ALU.add, ALU.max)
            m2q_t = ps1.tile([128, 16], f32, tag="m2")
            m2q = m2q_t[:, 0:NGRP]
            for g in range(NGRP):
                nc.tensor.matmul(m2q[:, g:g + 1], ysq_q[:, 128 * g:128 * (g + 1)],
                                 ones[:], start=True, stop=True)
            aq = work.tile([128, NGRP], f32, tag="aq")
            nc.vector.tensor_scalar(aq[:], m2q[:], 1.0 / 128, EPS, ALU.mult, ALU.add)
            rq = work.tile([128, NGRP], f32, tag="rq")
            nc.vector.reciprocal(rq[:], aq[:])
            istdq = work.tile([128, NGRP], f32, tag="istdq")
            nc.scalar.activation(istdq[:], rq[:], AF.Sqrt)
            qn = ps1.tile([128, 128], f32, tag="lv")
            for g in range(NGRP):
                nc.tensor.matmul(qn[:], zq[:, 128 * g:128 * (g + 1)], W2q[:],
                                 start=True, stop=True)
                nc.vector.scalar_tensor_tensor(qsb[:, g, :], qn[:], istdq[:, g:g + 1],
                                               b2qbc[:], ALU.mult, ALU.add)

            # ---- stage B: edge tiles ----
            nodep = ps1.tile([128, 64], f32, tag="nodep")
            for t in range(T_tiles):
                hd = io.tile([128, 1, 512], bf16, tag="hd")
                hs = io.tile([128, 1, 512], bf16, tag="hs")
                qg = io.tile([128, 1, 512], bf16, tag="qg")
                for gt, itab, tab, fdpr in ((hd, idd, hsb, 256), (hs, ids, hsb, 256),
                                            (qg, idq, qsb, 256)):
                    nc.gpsimd.dma_gather(
                        gt[:], tab[:], itab[:, 32 * t:32 * (t + 1)], 512, 512, 128,
                        transpose=True, sbuf_tokens_per_rank=128,
                        sbuf_free_dim_per_rank=fdpr)
                dft = io.tile([84, 512], bf16, tag="dft")
                nc.sync.dma_start(dft[:], dfT_d[:, 512 * t:512 * (t + 1)])
                St = io.tile([128, 4, 128], bf16, tag="St")
                nc.sync.dma_start(St[:], S_d[:, 4 * t:4 * (t + 1), :])
                rel = io.tile([128, 4, 4], f32, tag="rel")
                nc.sync.dma_start(rel[:], relN_d[:, 4 * t:4 * (t + 1), :])

                yk = psY.tile([128, 512], f32, tag="yk")
                yv = psY.tile([128, 512], f32, tag="yv")
                for y, c0 in ((yk, 0), (yv, 128)):
                    nc.tensor.matmul(y[:], W1a[:, c0:c0 + 128], dft[:], start=True, stop=False)
                    nc.tensor.matmul(y[:], W1b[:, c0:c0 + 128], hd[:, 0, :], start=False, stop=False)
                    nc.tensor.matmul(y[:], W1c[:, c0:c0 + 128], hs[:, 0, :], start=False, stop=True)
                zk = work.tile([128, 512], bf16, tag="zk")
                zv = work.tile([128, 512], bf16, tag="zv")
                ysqk = work.tile([128, 512], bf16, tag="ysqk")
                ysqv = work.tile([128, 512], bf16, tag="ysqv")
                nc.vector.tensor_scalar(zk[:], yk[:], b1k, 0.0, ALU.add, ALU.max)
                nc.scalar.activation(zv[:], yv[:], AF.Relu, bias=b1v)
                nc.scalar.activation(ysqk[:], yk[:], AF.Square, bias=b1k)
                nc.scalar.activation(ysqv[:], yv[:], AF.Square, bias=b1v)
                m2_t = ps1.tile([128, 16], f32, tag="m2")
                m2 = m2_t[:, 0:8]
                for c in range(4):
                    nc.tensor.matmul(m2[:, c:c + 1], ysqk[:, 128 * c:128 * (c + 1)],
                                     ones[:], start=True, stop=True)
                    nc.tensor.matmul(m2[:, 4 + c:5 + c], ysqv[:, 128 * c:128 * (c + 1)],
                                     ones[:], start=True, stop=True)
                ist_in = work.tile([128, 8], f32, tag="ist_in")
                # k-branch folds 1/sqrt(8): rsqrt(8*(m2/128+eps)) = rsqrt(m2/16+8eps)
                nc.vector.tensor_scalar(ist_in[:, 0:4], m2[:, 0:4], 1.0 / 16, 8.0 * EPS,
                                        ALU.mult, ALU.add)
                nc.vector.tensor_scalar(ist_in[:, 4:8], m2[:, 4:8], 1.0 / 128, EPS,
                                        ALU.mult, ALU.add)
                rec = work.tile([128, 8], f32, tag="rec")
                nc.vector.reciprocal(rec[:], ist_in[:])
                ist = work.tile([128, 8], f32, tag="ist")
                nc.scalar.activation(ist[:], rec[:], AF.Sqrt)

                M2k = ps1.tile([128, 512], f32, tag="M2k")
                nc.tensor.matmul(M2k[:], W2k[:], zk[:], start=True, stop=True)
                prod = work.tile([128, 512], bf16, tag="prod")
                nc.vector.scalar_tensor_tensor(prod[:], M2k[:], b2k, qg[:, 0, :],
                                               ALU.add, ALU.mult)
                lv = ps1.tile([128, 128], f32, tag="lv")
                for c in range(4):
                    nc.tensor.matmul(lv[:, 16 * c:16 * (c + 1)],
                                     prod[:, 128 * c:128 * (c + 1)], blk[:],
                                     start=True, stop=True)
                    nc.tensor.matmul(lv[:, 64 + 16 * c:80 + 16 * c],
                                     zv[:, 128 * c:128 * (c + 1)], W2v[:],
                                     start=True, stop=True)
                # head stage, per chunk (per-partition scalars)
                pay = work.tile([128, 4, 64], bf16, tag="pay")
                exf = work.tile([128, 4, 16], f32, tag="exf")
                evx = work.tile([128, 4, 16], f32, tag="evx")
                for c in range(4):
                    lmul = work.tile([128, 16], f32, tag="lmul")
                    nc.vector.tensor_scalar_mul(lmul[:], lv[:, 16 * c:16 * (c + 1)],
                                                ist[:, c:c + 1])
                    nc.scalar.activation(exf[:, c, :], lmul[:], AF.Exp)
                    nc.vector.tensor_copy(pay[:, c, 0:16], exf[:, c, :])
                    ev = work.tile([128, 16], f32, tag="ev")
                    nc.vector.scalar_tensor_tensor(
                        ev[:], lv[:, 64 + 16 * c:80 + 16 * c], ist[:, 4 + c:5 + c],
                        b2vbc[:, 16 * c:16 * (c + 1)], ALU.mult, ALU.add)
                    nc.vector.tensor_tensor(evx[:, c, :], ev[:], exf[:, c, :], ALU.mult)
                    for d in range(3):
                        nc.vector.tensor_scalar_mul(
                            pay[:, c, 16 * (d + 1):16 * (d + 2)], evx[:, c, :],
                            rel[:, c, d:d + 1])
                for c in range(4):
                    ch = 4 * t + c
                    nc.tensor.matmul(nodep[:], St[:, c, :], pay[:, c, :],
                                     start=(ch in grp_start), stop=(ch in grp_end))
                    if ch in grp_end:
                        g = grp_end[ch]
                        rden = work.tile([128, 16], f32, tag="rden")
                        nc.vector.reciprocal(rden[:], nodep[:, 0:16])
                        om = work.tile([128, 48], f32, tag="om")
                        for d in range(3):
                            nc.vector.tensor_tensor(om[:, 16 * d:16 * (d + 1)],
                                                    nodep[:, 16 * (d + 1):16 * (d + 2)],
                                                    rden[:], ALU.mult)
                            nc.vector.tensor_reduce(
                                osb[:, g, d:d + 1],
                                om[:, 16 * d:16 * (d + 1)],
                                mybir.AxisListType.X, ALU.add)
            oscaled = tabs.tile([128, NGRP, 4], f32, tag="osc")
            nc.vector.tensor_scalar_mul(oscaled[:], osb[:], 1.0 / H)
            nc.sync.dma_start(out_d[:], oscaled[:])

    if not nc.is_finalized():
        nc.finalize()
    return nc


def _prep(x, h, edge_attr, e_w, edge_index, weights):
    src = np.asarray(edge_index[0])
    dst = np.asarray(edge_index[1])
    core_of = dst // NLOC
    per_core_eids = []
    runs = np.zeros((NCORES, NGRP), dtype=np.int64)
    for c in range(NCORES):
        sel = np.nonzero(core_of == c)[0]
        order = np.argsort(dst[sel], kind="stable")
        eids = sel[order]
        per_core_eids.append(eids)
        ld = dst[eids] - NLOC * c
        g = ld // 128
        for gg in range(NGRP):
            runs[c, gg] = int((g == gg).sum())
    C = np.maximum(np.ceil(runs / 128).astype(np.int64).max(axis=0), 1)
    T_total = int(C.sum())
    T_total += (-T_total) % 4
    C[-1] += T_total - int(C.sum())
    E_pad = 128 * T_total
    # chunk -> group map, group start/end chunk ids
    grp_start, grp_end = {}, {}
    ch0 = 0
    for g in range(NGRP):
        grp_start[ch0] = g
        grp_end[ch0 + int(C[g]) - 1] = g
        ch0 += int(C[g])

    # gaussian features
    offs = np.linspace(0.0, RMAX, NG).astype(np.float32)
    coeff = np.float32(-0.5 / (offs[1] - offs[0]) ** 2)

    # weights (shared across cores)
    W1cat = np.concatenate([weights["xk_W1"], weights["xv_W1"]], axis=1)  # [340,256]
    Wk_c, b1k_c = _center(W1cat[:, :128], weights["xk_b1"])
    Wv_c, b1v_c = _center(W1cat[:, 128:], weights["xv_b1"])
    W1c_all = np.concatenate([Wk_c, Wv_c], axis=1)
    # fold LN gamma into W2 (exact when beta==0 contribution handled via b2)
    gk, bk = weights["xk_g"], weights["xk_bt"]
    gv, bv = weights["xv_g"], weights["xv_bt"]
    gq, bq = weights["xq_g"], weights["xq_bt"]
    W2k_f = weights["xk_W2"] * gk[:, None]
    b2k_f = weights["xk_b2"] + bk @ weights["xk_W2"]
    W2v_f = weights["xv_W2"] * gv[:, None]
    b2v_f = weights["xv_b2"] + bv @ weights["xv_W2"]
    W2q_f = weights["xq_W2"] * gq[:, None]
    b2q_f = weights["xq_b2"] + bq @ weights["xq_W2"]
    W1q_c, b1q_c = _center(weights["xq_W1"], weights["xq_b1"])

    shared = {
        "W1a": _bf(W1c_all[:84]), "W1b": _bf(W1c_all[84:212]),
        "W1c": _bf(W1c_all[212:340]),
        "W2k": _bf(W2k_f), "W2v": _bf(W2v_f), "W1q": _bf(W1q_c), "W2q": _bf(W2q_f),
        "ones": _bf(np.ones((128, 1))),
        "blk": _bf((np.arange(128)[:, None] // 8 == np.arange(16)[None, :])),
        "b2vbc": np.tile(b2v_f[None, :], (128, 4)).astype(np.float32),
        "b2qbc": np.tile(b2q_f[None, :], (128, 1)).astype(np.float32),
    }
    bias = np.zeros((128, 8), np.float32)
    bias[:, 0] = b1k_c
    bias[:, 1] = b1v_c
    bias[:, 2] = b1q_c
    bias[:, 3] = b2k_f
    shared["bias"] = bias

    hpad = np.zeros((HSTRIPES * 128, D), np.float32)
    hpad[:N] = h
    hsb = _bf(hpad.reshape(HSTRIPES, 128, D).transpose(1, 0, 2))

    in_maps = []
    for c in range(NCORES):
        eids = per_core_eids[c]
        ld = dst[eids] - NLOC * c
        g = ld // 128
        # build padded edge arrays
        e_dst = np.zeros(E_pad, np.int64)
        e_src = np.zeros(E_pad, np.int64)
        e_ldq = np.zeros(E_pad, np.int64)
        e_rel = np.zeros((E_pad, 4), np.float32)
        e_df = np.zeros((E_pad, 84), np.float32)
        S = np.zeros((E_pad, 128), np.float32)
        pos = 0
        ch0 = 0
        for gg in range(NGRP):
            idx = eids[g == gg]
            k = len(idx)
            s0 = 128 * ch0
            if k:
                e_dst[s0:s0 + k] = dst[idx]
                e_src[s0:s0 + k] = src[idx]
                e_ldq[s0:s0 + k] = dst[idx] - NLOC * c
                rl = x[dst[idx]] - x[src[idx]]
                e_rel[s0:s0 + k, :3] = rl
                dist = np.linalg.norm(rl, axis=1, keepdims=True)
                gauss = np.exp(coeff * (dist - offs[None, :]) ** 2)
                ea = edge_attr[idx]
                e_df[s0:s0 + k, :4] = ea
                e_df[s0:s0 + k, 4:] = (ea[:, :, None] * gauss[:, None, :]).reshape(k, 80)
                S[s0 + np.arange(k), (dst[idx] - NLOC * c) - 128 * gg] = 1.0
            ch0 += int(C[gg])
            pos += k
        hl = np.zeros((NGRP * 128, D), np.float32)
        hl[:NLOC] = h[NLOC * c:NLOC * (c + 1)]
        m = {
            "hsb": hsb,
            "hlocT": _bf(hl.T),
            "idd": _wrap_idx(e_dst), "ids": _wrap_idx(e_src), "idq": _wrap_idx(e_ldq),
            "dfT": _bf(e_df.T),
            "relN": e_rel.reshape(E_pad // 128, 128, 4).transpose(1, 0, 2).astype(np.float32).copy(),
            "S": _bf(S.reshape(T_total, 128, 128).transpose(1, 0, 2)),
        }
        m.update(shared)
        in_maps.append(m)
    return in_maps, T_total, grp_start, grp_end


def kernel(**inputs):
    wnames = [k for k in inputs if k.startswith(("xk_", "xv_", "xq_"))]
    weights = {k: np.asarray(inputs[k], np.float32) for k in wnames}
    in_maps, T_total, grp_start, grp_end = _prep(
        np.asarray(inputs["x"], np.float32), np.asarray(inputs["h"], np.float32),
        np.asarray(inputs["edge_attr"], np.float32), inputs["e_w"],
        np.asarray(inputs["edge_index"]), weights)
    key = (T_total, tuple(sorted(grp_start.items())), tuple(sorted(grp_end.items())))
    if key not in _cache:
        _cache[key] = _build_program(T_total, grp_start, grp_end)
    nc = _cache[key]
    res = run_bass_kernel_spmd(nc, in_maps, core_ids=list(range(NCORES)))
    outs = []
    for c in range(NCORES):
        o = res.results[c]["o"]  # [128, NGRP, 4] -> node g*128+p
        outs.append(o.transpose(1, 0, 2).reshape(NGRP * 128, 4)[:NLOC, :3])
    out = np.concatenate(outs, axis=0)
    return out.astype(np.float32)
